# revision 7
# baseline (speedup 1.0000x reference)
"""Trainium2 Bass kernel for CepstralBlock: causal depthwise conv along D
(K=4, per-channel weights) followed by a 128x128 Linear.

v3 schedule, built from NTFF traces of the 64-68us baselines:
  * All consts in ONE [C, 648] bf16 DMA issued first on the SP ring
    (1296B-per-partition descriptors land in <1us; the baseline's three
    small const DMAs trickled until ~14us and gated conv + drains).
    Cols 0-639: W_0..W_3 | W; 640-643: tap weights; 644: bias (cast to
    f32 on the DVE for the ACT drains).
  * 8 PE warmup matmuls: the HAM clock gate needs ~3.5us of sustained PE
    activity to lift the PE from 1.2 to 2.4 GHz; with fewer warmups the
    first ~12 real matmuls run at half rate (measured v2 regression).
  * PSUM as 4 x [C, 2S] tiles (was 2 x [C, 4S]): the tail previously
    serialized at ~3.4us/chunk on the matmul->drain ping-pong of a
    2-buffer PSUM; 4 sub-chunks in flight halve that.
  * Factored (DVE) conv: 5 uniform 4-slice groups (b0 d4-12, b1 d4-12,
    b2 d8-12).  Batch 3 is pure fold so the tail never waits on the DVE.
  * Stores ride the SP ring (issued after the 16 loads; the ACT engine
    does nothing but drains).  The last stores are split for receipt
    overlap.  The final conv-gated sub-chunk drains on the DVE, which is
    free right after its conv group.

Layout: host pre-transposes x to channel-major [C, B, D, S] so C=128 sits
on the SBUF partition axis (the matmul contraction axis).  Data-parallel
over H (64 -> 8 per core), 8 NeuronCores, no collectives.  bf16 compute,
f32 PSUM accumulation, bf16 output.
"""

import sys
import types

sys.path.insert(0, "/opt/trn_rl_repo")

import numpy as np
import ml_dtypes

# Problem shapes (hardcoded; kernel.py must be self-contained).
B = 4
D = 16
H = 64
WD = 64
C = 128
KS = 4
NCORES = 8
HSH = H // NCORES          # 8 H-rows per core
S = HSH * WD               # 512 spatial positions per (b, d) slice

DCH = 4                    # depth slices per x chunk
NCH = D // DCH
PCH = 4                    # depth slices per store chunk (2 PSUM subtiles)
NCONST = 2048              # wk (640) + wt (4) + bias (1) + pad to 4KB descs

# Factored (DVE-conv) groups: uniform 4-slice runs.  Emission order of
# this list IS the DVE queue order.
CONV_GROUPS = [
    (0, 4, 12),
    (1, 5, 12),
    (2, 8, 12),
]
# Per-batch load order of x chunks on the SP ring.
LOAD_ORDER = [0, 1, 3, 2]
# (batch, chunk) emission order for PE matmuls / drains / stores, sorted
# by expected readiness (x arrival for fold, conv completion for fact).
GLOBAL_SEQ = [(0, 0), (0, 3), (1, 0), (1, 3), (2, 0), (0, 1), (2, 1),
              (0, 2), (2, 3), (3, 0), (3, 1), (3, 3), (1, 1), (3, 2),
              (1, 2), (2, 2)]
# PSUM subtiles whose drain rides the DVE instead of ACT (only safe after
# the DVE's conv queue is finished).
DVE_DRAINS = set()
# Late chunks drain as ACT [0:2S] || DVE [2S:4S] halves (the DVE conv
# queue is done by then, and halving the drain latency unblocks the
# PSUM ping-pong in the tail).
SPLIT_DRAINS = {(3, 3), (3, 2), (1, 2), (2, 2)}
# Final chunks whose stores are split so HBM receipts overlap.
SPLIT_STORES = {(1, 2), (2, 2)}
NWARM = 10

# Set by test.py to run with NTFF profiling and stash exec time here.
PROFILE = False
NPROF = 8          # traced runs when PROFILE; min exec_time_ns is reported
LAST_EXEC_NS = None
LAST_EXEC_ALL = None
LAST_RESULT = None

_graph_cache = {}


def _install_ntff_hook():
    """Provide antenv.axon_hooks + register the NTFF profile hook if the
    image's antenv package lacks it (needed for trace=True under axon)."""
    try:
        from antenv import axon_hooks  # noqa: F401
        return
    except ImportError:
        pass
    try:
        import antenv
        from trn_agent_boot.trn_boot import _ntff_profile_via_ctypes
    except ImportError:
        return
    mod = types.ModuleType("antenv.axon_hooks")
    mod._hook = None

    def set_axon_ntff_profile_hook(h):
        mod._hook = h

    def get_axon_ntff_profile_hook():
        return mod._hook

    mod.set_axon_ntff_profile_hook = set_axon_ntff_profile_hook
    mod.get_axon_ntff_profile_hook = get_axon_ntff_profile_hook
    sys.modules["antenv.axon_hooks"] = mod
    antenv.axon_hooks = mod
    mod.set_axon_ntff_profile_hook(
        _ntff_profile_via_ctypes("/opt/axon/libaxon_pjrt.so")
    )


def _build_graph():
    import concourse.mybir as mybir
    from concourse import bacc
    from concourse.tile import TileContext

    nc = bacc.Bacc("TRN2", target_bir_lowering=False, debug=False,
                   num_devices=NCORES)
    xt = nc.declare_dram_parameter("xt", [C, B, D, S], mybir.dt.bfloat16,
                                   isOutput=False)
    # packed consts: W_0..W_3 | W | wt | bias | pad   [C, 648] bf16
    ck = nc.declare_dram_parameter("ck", [C, NCONST], mybir.dt.bfloat16,
                                   isOutput=False)
    out = nc.declare_dram_parameter("out", [C, B, D, S], mybir.dt.bfloat16,
                                    isOutput=True)

    add = mybir.AluOpType.add
    ident = mybir.ActivationFunctionType.Identity

    fact = {b: set() for b in range(B)}
    for b, g0, g1 in CONV_GROUPS:
        fact[b].update(range(g0, g1))

    with TileContext(nc) as tc:
        with (
            tc.tile_pool(name="consts", bufs=1) as cpool,
            tc.tile_pool(name="xin", bufs=B * NCH) as xpool,
            tc.tile_pool(name="prod", bufs=5) as ppool,
            tc.tile_pool(name="yconv", bufs=3) as ypool,
            tc.tile_pool(name="ostage", bufs=14) as opool,
            tc.tile_pool(name="ps", bufs=2, space="PSUM") as pspool,
        ):
            # One packed const DMA, first on the SP ring; x loads right
            # behind it.
            ck_sb = cpool.tile([C, NCONST], mybir.dt.bfloat16)
            nc.sync.dma_start(out=ck_sb[:], in_=ck[:])
            # Warmup source memset FIRST on the DVE queue: the wt_f cast
            # below waits on the const DMA, and the PE warmups must not
            # inherit that wait (the HAM clock ramp needs the PE busy
            # continuously from the preamble until real matmuls start).
            warm_src = cpool.tile([C, S], mybir.dt.bfloat16)
            nc.vector.memset(warm_src[:], 0.0)
            # f32 tap weights + bias for the DVE/ACT ops (cast from the
            # bf16 cols 640-644; scalar operands must be f32).
            wt_f = cpool.tile([C, KS + 1], mybir.dt.float32)
            nc.vector.tensor_scalar_add(wt_f[:], ck_sb[:, 640:645], 0.0)
            bias_f = wt_f[:, KS:KS + 1]

            def wmat(k):
                return ck_sb[:, k * C:(k + 1) * C]

            def wtap(k):
                return wt_f[:, k:k + 1]

            # Keep the PE busy while the first x chunks stream in, so the
            # HAM clock gate lifts the PE clock before the real matmuls
            # start (needs ~3.5us of sustained activity).  PSUM is fully
            # claimed by the 4 [C, 2S] subtiles, so the warmups write
            # halves of pool tiles (PE-queue order keeps the reuse safe).
            for i in range(2):
                wtile = pspool.tile([C, PCH * S], mybir.dt.float32, tag="ps",
                                    name=f"warm_{i}")
                for q in range(NWARM // 2):
                    nc.tensor.matmul(wtile[:, (q % PCH) * S:
                                            (q % PCH + 1) * S],
                                     warm_src[:, 0:C], warm_src[:],
                                     start=True, stop=True)

            # All x loads up front on the SP ring.
            xch = {}
            for b in range(B):
                for i in LOAD_ORDER:
                    t = xpool.tile([C, DCH * S], mybir.dt.bfloat16, tag="xh",
                                   name=f"xh_{b}_{i}")
                    nc.sync.dma_start(
                        out=t[:],
                        in_=xt[:, b, i * DCH:(i + 1) * DCH].rearrange(
                            "c d s -> c (d s)"),
                    )
                    xch[(b, i)] = t

            def x_view(b, d0, d1):
                """[C, (d1-d0)*S] view; [d0, d1) must live in one chunk."""
                i = d0 // DCH
                assert (d1 - 1) // DCH == i, (d0, d1)
                j = d0 % DCH
                return xch[(b, i)][:, j * S:(j + d1 - d0) * S]

            def conv_products(b, g0, g1):
                """Per-tap products for slices [g0, g1) via fast single-src
                tensor_scalar_mul; reads split at x-chunk boundaries."""
                prods = []
                n = g1 - g0
                for k in range(KS):
                    tk = ppool.tile([C, n * S], mybir.dt.bfloat16,
                                    tag="pp", name=f"pp_{b}_{g0}_{k}")
                    a = g0 - k
                    while a < g1 - k:
                        bnd = min(g1 - k, (a // DCH + 1) * DCH)
                        o = (a + k - g0) * S
                        nc.vector.tensor_scalar_mul(
                            tk[:, o:o + (bnd - a) * S],
                            x_view(b, a, bnd),
                            wtap(k))
                        a = bnd
                    prods.append(tk)
                return prods

            # Pass 1 -- all DVE conv work in CONV_GROUPS order.  The DVE
            # queue must never wait on anything but x-chunk arrival.
            ysall = {}
            for b, g0, g1 in CONV_GROUPS:
                n = (g1 - g0) * S
                p = conv_products(b, g0, g1)
                nc.vector.tensor_tensor(p[1][:, 0:n], p[1][:, 0:n],
                                        p[0][:, 0:n], add)
                nc.vector.tensor_tensor(p[3][:, 0:n], p[3][:, 0:n],
                                        p[2][:, 0:n], add)
                y = ypool.tile([C, n], mybir.dt.bfloat16, tag="yc",
                               name=f"yc_{b}_{g0}")
                # final add split at the d=8 chunk boundary so the first
                # store chunk's slices release before the whole group.
                cuts = [0] + ([  (8 - g0) * S] if g0 < 8 < g1 else []) + [n]
                for c0, c1 in zip(cuts, cuts[1:]):
                    nc.vector.tensor_tensor(y[:, c0:c1], p[3][:, c0:c1],
                                            p[1][:, c0:c1], add)
                for d in range(g0, g1):
                    ysall[(b, d)] = y[:, (d - g0) * S:(d - g0 + 1) * S]

            # Pass 2 -- PE matmuls + PSUM drains (2-slice subtiles) +
            # stores (4-slice chunks) in readiness order.
            for b, i in GLOBAL_SEQ:
                ob = opool.tile([C, PCH * S], mybir.dt.bfloat16,
                                tag="ob", name=f"ob_{b}_{i}")
                psc = pspool.tile([C, PCH * S], mybir.dt.float32,
                                  tag="ps", name=f"ps_{b}_{i}")
                for d in range(i * PCH, (i + 1) * PCH):
                    pq = psc[:, (d % PCH) * S:(d % PCH + 1) * S]
                    if d in fact[b]:
                        nc.tensor.matmul(pq, wmat(KS), ysall[(b, d)],
                                         start=True, stop=True)
                    else:
                        ks = [k for k in range(KS) if d - k >= 0]
                        for k in ks:
                            nc.tensor.matmul(
                                pq, wmat(k),
                                x_view(b, d - k, d - k + 1),
                                start=(k == 0), stop=(k == ks[-1]))
                if (b, i) in SPLIT_DRAINS:
                    nc.scalar.activation(ob[:, 0:2 * S], psc[:, 0:2 * S],
                                         ident, bias=bias_f[:, 0:1],
                                         scale=1.0)
                    nc.vector.tensor_scalar_add(ob[:, 2 * S:4 * S],
                                                psc[:, 2 * S:4 * S],
                                                bias_f[:, 0:1])
                else:
                    nc.scalar.activation(ob[:], psc[:], ident,
                                         bias=bias_f[:, 0:1], scale=1.0)
                # Stores ride the SWDGE (gpsimd) queue row: it round-robins
                # with the SP load row at the SDMA level, so loads and
                # stores overlap; the ACT/SP sequencers stay free.
                if (b, i) in SPLIT_STORES:
                    for hh in range(2):
                        nc.gpsimd.dma_start(
                            out=out[:, b, i * PCH + hh * 2:
                                    i * PCH + hh * 2 + 2].rearrange(
                                "c d s -> c (d s)"),
                            in_=ob[:, hh * 2 * S:(hh * 2 + 2) * S],
                        )
                else:
                    nc.gpsimd.dma_start(
                        out=out[:, b, i * PCH:(i + 1) * PCH].rearrange(
                            "c d s -> c (d s)"),
                        in_=ob[:],
                    )
    nc.compile()
    return nc


def _get_graph():
    if "nc" not in _graph_cache:
        _graph_cache["nc"] = _build_graph()
    return _graph_cache["nc"]


def kernel(x, kernel, W, b):
    global LAST_EXEC_NS, LAST_RESULT
    from concourse.bass_utils import run_bass_kernel_spmd

    nc = _get_graph()

    x = np.asarray(x, np.float32)
    kernel = np.asarray(kernel, np.float32)
    W = np.asarray(W, np.float32)
    b = np.asarray(b, np.float32)

    # Host precompute: fold the depthwise filter into 4 Linear weights,
    # append the plain W, the raw tap weights and the bias.
    w_full = np.tile(kernel, (C // kernel.shape[0], 1))          # [C, KS]
    ck = np.zeros((C, NCONST), np.float32)
    for k in range(KS):
        ck[:, k * C:(k + 1) * C] = w_full[:, k:k + 1] * W
    ck[:, KS * C:(KS + 1) * C] = W
    ck[:, 640:644] = w_full
    ck[:, 644] = b
    ck = ck.astype(ml_dtypes.bfloat16)

    # Channel-major transpose + H-shard + bf16.
    xbf = x.astype(ml_dtypes.bfloat16)
    xtr = np.transpose(xbf, (4, 0, 1, 2, 3))                     # [C,B,D,H,W]
    in_maps = []
    for i in range(NCORES):
        shard = np.ascontiguousarray(
            xtr[:, :, :, i * HSH:(i + 1) * HSH, :]
        ).reshape(C, B, D, S)
        in_maps.append({"xt": shard, "ck": ck})

    global LAST_EXEC_ALL
    core_ids = list(range(NCORES))
    res = None
    if PROFILE:
        _install_ntff_hook()
        try:
            # Warm run first: the NEFF compile on a cold cache must not
            # happen inside the NTFF capture window.
            run_bass_kernel_spmd(nc, in_maps, core_ids=core_ids)
            times = []
            for _ in range(max(1, NPROF)):
                res = run_bass_kernel_spmd(nc, in_maps, core_ids=core_ids,
                                           trace=True)
                times.append(res.exec_time_ns)
            LAST_EXEC_ALL = times
        except Exception as e:
            print(f"profile run failed ({type(e).__name__}: {e}); "
                  "falling back to non-traced run", file=sys.stderr)
            res = None
    if res is None:
        res = run_bass_kernel_spmd(nc, in_maps, core_ids=core_ids)
        LAST_EXEC_NS = res.exec_time_ns
    else:
        LAST_EXEC_NS = min(t for t in LAST_EXEC_ALL if t is not None)
    LAST_RESULT = res

    # Gather: shard_i[o, b, d, h*WD + w] -> full[b, d, i*HSH + h, w, o]
    o = np.stack([np.asarray(res.results[i]["out"]) for i in range(NCORES)],
                 axis=0).astype(np.float32)
    o = o.reshape(NCORES, C, B, D, HSH, WD)
    o = np.transpose(o, (2, 3, 0, 4, 5, 1)).reshape(B, D, H, WD, C)
    return np.ascontiguousarray(o)


# revision 8
# speedup vs baseline: 1.0420x; 1.0420x over previous
"""Trainium2 Bass kernel for CepstralBlock: causal depthwise conv along D
(K=4, per-channel weights) followed by a 128x128 Linear.

v7 = the proven v1 schedule with three measured bottlenecks fixed:
  * Consts: ONE [C, 2048] bf16 padded DMA (4KB-per-partition descriptors,
    line rate) on the ACT ring instead of three small-descriptor DMAs
    that crawled at ~23 GB/s until 13.2us and gated the first matmul.
    Tap weights + bias are cast to f32 on the DVE (scalar operands must
    be f32).
  * Stores ride the SWDGE (gpsimd) queue row: the SDMA round-robins it
    against the SP load row, so loads/stores overlap, and the ACT
    sequencer sheds ~13us of DMA-issue work (it was the busiest engine).
  * F=24 -> 20 factored slices ((6,11) per batch): the DVE conv ends
    ~6us earlier so the final factored chunks stop gating the tail.

Hybrid engine split:
  * fold slices   : conv folded into the matmul -- out_d = sum_k x_{d-k}@W_k
    with W_k = diag(w_k) @ W, 1-4 PSUM-accumulated matmuls per slice.
  * factored slices: conv on the DVE -- per-tap products via
    tensor_scalar_mul (fast single-src mode), pairwise sums via
    tensor_tensor ADD (bf16 2x mode) -- then ONE matmul per slice.
  * PSUM -> SBUF + bias rides the ACT engine as ONE activation(Identity)
    per [C, 4*S] PSUM chunk; one tail chunk drains on the DVE in
    parallel.

Layout: host pre-transposes x to channel-major [C, B, D, S] so C=128 sits
on the SBUF partition axis (the matmul contraction axis).  Data-parallel
over H (64 -> 8 per core), 8 NeuronCores, no collectives.  bf16 compute,
f32 PSUM accumulation, bf16 output.
"""

import sys
import types

sys.path.insert(0, "/opt/trn_rl_repo")

import numpy as np
import ml_dtypes

# Problem shapes (hardcoded; kernel.py must be self-contained).
B = 4
D = 16
H = 64
WD = 64
C = 128
KS = 4
NCORES = 8
HSH = H // NCORES          # 8 H-rows per core
S = HSH * WD               # 512 spatial positions per (b, d) slice

DCH = 4                    # depth slices per x/psum/out chunk
NCH = D // DCH
PCH = 4                    # depth slices per psum/act/store chunk
NCONST = 2048              # wk (640) + wt (4) + bias (1) + pad to 4KB descs

# Factored (DVE-conv) slice groups per batch (contiguous runs).
FACT_GROUPS = {
    0: [(6, 11)],
    1: [(6, 11)],
    2: [(6, 11)],
    3: [(6, 11)],
}
# chunk 1 first: the DVE conv group's first product ops read it
LOAD_ORDER = [1, 0, 2, 3]
# Global (batch, chunk) emission order for PE/ACT/stores: fold chunks of
# batch b+1 run before the factored chunks of batch b, giving the DVE a
# full batch-period of lead time.
GLOBAL_SEQ = [(0, 0), (0, 3), (1, 0), (1, 3), (0, 1), (0, 2),
              (2, 0), (2, 3), (1, 1), (1, 2), (3, 0), (3, 3),
              (2, 1), (2, 2), (3, 1), (3, 2)]
NWARM = 8

# Set by test.py to run with NTFF profiling and stash exec time here.
PROFILE = False
NPROF = 8          # traced runs when PROFILE; min exec_time_ns is reported
LAST_EXEC_NS = None
LAST_EXEC_ALL = None
LAST_RESULT = None

_graph_cache = {}


def _install_ntff_hook():
    """Provide antenv.axon_hooks + register the NTFF profile hook if the
    image's antenv package lacks it (needed for trace=True under axon)."""
    try:
        from antenv import axon_hooks  # noqa: F401
        return
    except ImportError:
        pass
    try:
        import antenv
        from trn_agent_boot.trn_boot import _ntff_profile_via_ctypes
    except ImportError:
        return
    mod = types.ModuleType("antenv.axon_hooks")
    mod._hook = None

    def set_axon_ntff_profile_hook(h):
        mod._hook = h

    def get_axon_ntff_profile_hook():
        return mod._hook

    mod.set_axon_ntff_profile_hook = set_axon_ntff_profile_hook
    mod.get_axon_ntff_profile_hook = get_axon_ntff_profile_hook
    sys.modules["antenv.axon_hooks"] = mod
    antenv.axon_hooks = mod
    mod.set_axon_ntff_profile_hook(
        _ntff_profile_via_ctypes("/opt/axon/libaxon_pjrt.so")
    )


def _build_graph():
    import concourse.mybir as mybir
    from concourse import bacc
    from concourse.tile import TileContext

    nc = bacc.Bacc("TRN2", target_bir_lowering=False, debug=False,
                   num_devices=NCORES)
    xt = nc.declare_dram_parameter("xt", [C, B, D, S], mybir.dt.bfloat16,
                                   isOutput=False)
    # packed consts: W_0..W_3 | W | wt | bias | pad   [C, 2048] bf16
    ck = nc.declare_dram_parameter("ck", [C, NCONST], mybir.dt.bfloat16,
                                   isOutput=False)
    out = nc.declare_dram_parameter("out", [C, B, D, S], mybir.dt.bfloat16,
                                    isOutput=True)

    add = mybir.AluOpType.add
    ident = mybir.ActivationFunctionType.Identity

    fact = {b: set() for b in range(B)}
    for b, groups in FACT_GROUPS.items():
        for g0, g1 in groups:
            fact[b].update(range(g0, g1))

    with TileContext(nc) as tc:
        with (
            tc.tile_pool(name="consts", bufs=1) as cpool,
            tc.tile_pool(name="xin", bufs=B * NCH) as xpool,
            tc.tile_pool(name="prod", bufs=6) as ppool,
            tc.tile_pool(name="yconv", bufs=4) as ypool,
            tc.tile_pool(name="ostage", bufs=14) as opool,
            tc.tile_pool(name="ps", bufs=2, space="PSUM") as pspool,
        ):
            # One padded const DMA on the ACT ring (4KB descriptors, so it
            # lands at line rate in parallel with the x loads on SP).
            ck_sb = cpool.tile([C, NCONST], mybir.dt.bfloat16)
            nc.scalar.dma_start(out=ck_sb[:], in_=ck[:])

            # Warmup source memset FIRST on the DVE queue (the wt_f cast
            # below waits on the const DMA and must not delay the PE
            # warmups -- the HAM clock ramp needs the PE busy from the
            # preamble on).
            warm_src = cpool.tile([C, S], mybir.dt.bfloat16)
            nc.vector.memset(warm_src[:], 0.0)
            # f32 tap weights + bias (scalar operands must be f32).
            wt_f = cpool.tile([C, KS + 1], mybir.dt.float32)
            nc.vector.tensor_scalar_add(wt_f[:], ck_sb[:, 640:645], 0.0)
            bias_f = wt_f[:, KS:KS + 1]

            def wmat(k):
                return ck_sb[:, k * C:(k + 1) * C]

            # Keep the PE busy while the first x chunks stream in, so the
            # HAM clock gate flips to 2.4 GHz before the real matmuls
            # start.  PSUM is fully claimed by the 2 [C, 4S] chunk tiles,
            # so the warmups write quarters of pool tiles (PE-queue order
            # keeps the reuse safe).
            for i in range(2):
                wtile = pspool.tile([C, PCH * S], mybir.dt.float32, tag="ps",
                                    name=f"warm_{i}")
                for q in range(NWARM // 2):
                    nc.tensor.matmul(wtile[:, (q % PCH) * S:
                                            (q % PCH + 1) * S],
                                     warm_src[:, 0:C], warm_src[:],
                                     start=True, stop=True)

            # All x loads up front on the SP ring.
            xch = {}
            for b in range(B):
                for i in LOAD_ORDER:
                    t = xpool.tile([C, DCH * S], mybir.dt.bfloat16, tag="xh",
                                   name=f"xh_{b}_{i}")
                    nc.sync.dma_start(
                        out=t[:],
                        in_=xt[:, b, i * DCH:(i + 1) * DCH].rearrange(
                            "c d s -> c (d s)"),
                    )
                    xch[(b, i)] = t

            def x_view(b, d0, d1):
                """[C, (d1-d0)*S] view; [d0, d1) must live in one chunk."""
                i = d0 // DCH
                assert (d1 - 1) // DCH == i, (d0, d1)
                j = d0 % DCH
                return xch[(b, i)][:, j * S:(j + d1 - d0) * S]

            def conv_products(b, g0, g1):
                """Per-tap products for slices [g0, g1) via fast single-src
                tensor_scalar_mul; reads split at x-chunk boundaries."""
                prods = []
                n = g1 - g0
                for k in range(KS):
                    tk = ppool.tile([C, n * S], mybir.dt.bfloat16,
                                    tag="pp", name=f"pp_{b}_{g0}_{k}")
                    a = g0 - k
                    while a < g1 - k:
                        bnd = min(g1 - k, (a // DCH + 1) * DCH)
                        o = (a + k - g0) * S
                        nc.vector.tensor_scalar_mul(
                            tk[:, o:o + (bnd - a) * S],
                            x_view(b, a, bnd),
                            wt_f[:, k:k + 1])
                        a = bnd
                    prods.append(tk)
                return prods

            # Pass 1 -- all DVE conv work, batch-major.  The DVE is the
            # saturated engine; its queue must never wait on anything but
            # x-chunk arrival.
            ysall = {}
            for b in range(B):
                for g0, g1 in FACT_GROUPS.get(b, []):
                    n = (g1 - g0) * S
                    p = conv_products(b, g0, g1)
                    nc.vector.tensor_tensor(p[1][:, 0:n], p[1][:, 0:n],
                                            p[0][:, 0:n], add)
                    nc.vector.tensor_tensor(p[3][:, 0:n], p[3][:, 0:n],
                                            p[2][:, 0:n], add)
                    y = ypool.tile([C, n], mybir.dt.bfloat16, tag="yc",
                                   name=f"yc_{b}_{g0}")
                    nc.vector.tensor_tensor(y[:, 0:n], p[3][:, 0:n],
                                            p[1][:, 0:n], add)
                    for d in range(g0, g1):
                        ysall[(b, d)] = y[:, (d - g0) * S:(d - g0 + 1) * S]

            # Pass 2 -- PE matmuls + PSUM drain + stores in a cross-batch
            # interleaved order: each batch's factored chunks are deferred
            # one batch so the PE never head-of-line blocks waiting for the
            # DVE conv output of the batch it is currently folding.
            for b, i in GLOBAL_SEQ:
                psc = pspool.tile([C, PCH * S], mybir.dt.float32,
                                  tag="ps", name=f"ps_{b}_{i}")
                for d in range(i * PCH, (i + 1) * PCH):
                    pq = psc[:, (d % PCH) * S:(d % PCH + 1) * S]
                    if d in fact[b]:
                        nc.tensor.matmul(
                            pq, wmat(KS), ysall[(b, d)],
                            start=True, stop=True)
                    else:
                        ks = [k for k in range(KS) if d - k >= 0]
                        for k in ks:
                            nc.tensor.matmul(
                                pq, wmat(k),
                                x_view(b, d - k, d - k + 1),
                                start=(k == 0), stop=(k == ks[-1]))
                ob = opool.tile([C, PCH * S], mybir.dt.bfloat16,
                                tag="ob", name=f"ob_{b}_{i}")
                if b == B - 1 and i == 1:
                    # parallel tail drain: DVE takes this chunk while the
                    # ACT engine handles the final factored chunk
                    nc.vector.tensor_scalar_add(ob[:], psc[:],
                                                bias_f[:, 0:1])
                else:
                    nc.scalar.activation(ob[:], psc[:], ident,
                                         bias=bias_f[:, 0:1], scale=1.0)
                # Stores ride the SWDGE (gpsimd) queue row: the SDMA
                # round-robins it against the SP load row, so loads and
                # stores overlap and the ACT/SP sequencers stay free.
                if b == B - 1 and i in (1, 2):
                    # split the final stores so their HBM completion
                    # receipts overlap
                    for hh in range(2):
                        nc.gpsimd.dma_start(
                            out=out[:, b, i * PCH + hh * 2:
                                    i * PCH + hh * 2 + 2].rearrange(
                                "c d s -> c (d s)"),
                            in_=ob[:, hh * 2 * S:(hh * 2 + 2) * S],
                        )
                else:
                    nc.gpsimd.dma_start(
                        out=out[:, b, i * PCH:(i + 1) * PCH].rearrange(
                            "c d s -> c (d s)"),
                        in_=ob[:],
                    )
    nc.compile()
    return nc


def _get_graph():
    if "nc" not in _graph_cache:
        _graph_cache["nc"] = _build_graph()
    return _graph_cache["nc"]


def kernel(x, kernel, W, b):
    global LAST_EXEC_NS, LAST_RESULT
    from concourse.bass_utils import run_bass_kernel_spmd

    nc = _get_graph()

    x = np.asarray(x, np.float32)
    kernel = np.asarray(kernel, np.float32)
    W = np.asarray(W, np.float32)
    b = np.asarray(b, np.float32)

    # Host precompute: fold the depthwise filter into 4 Linear weights,
    # append the plain W, the raw tap weights and the bias.
    w_full = np.tile(kernel, (C // kernel.shape[0], 1))          # [C, KS]
    ck = np.zeros((C, NCONST), np.float32)
    for k in range(KS):
        ck[:, k * C:(k + 1) * C] = w_full[:, k:k + 1] * W
    ck[:, KS * C:(KS + 1) * C] = W
    ck[:, 640:644] = w_full
    ck[:, 644] = b
    ck = ck.astype(ml_dtypes.bfloat16)

    # Channel-major transpose + H-shard + bf16.
    xbf = x.astype(ml_dtypes.bfloat16)
    xtr = np.transpose(xbf, (4, 0, 1, 2, 3))                     # [C,B,D,H,W]
    in_maps = []
    for i in range(NCORES):
        shard = np.ascontiguousarray(
            xtr[:, :, :, i * HSH:(i + 1) * HSH, :]
        ).reshape(C, B, D, S)
        in_maps.append({"xt": shard, "ck": ck})

    global LAST_EXEC_ALL
    core_ids = list(range(NCORES))
    res = None
    if PROFILE:
        _install_ntff_hook()
        try:
            # Warm run first: the NEFF compile on a cold cache must not
            # happen inside the NTFF capture window.
            run_bass_kernel_spmd(nc, in_maps, core_ids=core_ids)
            times = []
            for _ in range(max(1, NPROF)):
                res = run_bass_kernel_spmd(nc, in_maps, core_ids=core_ids,
                                           trace=True)
                times.append(res.exec_time_ns)
            LAST_EXEC_ALL = times
        except Exception as e:
            print(f"profile run failed ({type(e).__name__}: {e}); "
                  "falling back to non-traced run", file=sys.stderr)
            res = None
    if res is None:
        res = run_bass_kernel_spmd(nc, in_maps, core_ids=core_ids)
        LAST_EXEC_NS = res.exec_time_ns
    else:
        LAST_EXEC_NS = min(t for t in LAST_EXEC_ALL if t is not None)
    LAST_RESULT = res

    # Gather: shard_i[o, b, d, h*WD + w] -> full[b, d, i*HSH + h, w, o]
    o = np.stack([np.asarray(res.results[i]["out"]) for i in range(NCORES)],
                 axis=0).astype(np.float32)
    o = o.reshape(NCORES, C, B, D, HSH, WD)
    o = np.transpose(o, (2, 3, 0, 4, 5, 1)).reshape(B, D, H, WD, C)
    return np.ascontiguousarray(o)
